# revision 3
# baseline (speedup 1.0000x reference)
"""Trainium2 Bass kernel for nn_GCLSTMModel_48868137894020 (v4).

Algebraic reduction (H0 = C0 = 0 kills the cheb convs, the forget gate,
and peep[0/1]); per layer (d = 140 then 280), X = input:
  I = sigmoid(X @ W[0] + cb[0] + b[0])
  T = tanh   (X @ W[2] + cb[2] + b[2])
  C = I * T
  O = sigmoid(X @ W[3] + cb[3] + b[3] + peep[2] * C)
  X' = relu(O * tanh(C)) == max(tanh(C), 0) * O
then out = relu(X'' @ fc_w + fc_b).

v4 on top of v3's two-ring DMA plan:
  - sigmoid(x) = (1 + tanh(x/2)) / 2: the I-gate weights are halved on
    the host, and I+T land interleaved per-chunk in ONE psum bank, so
    one tanh ACTIVATE per chunk covers both gates (the ACT engine was
    the tail bottleneck).  C' := 2C = (1 + tI) * tT is one DVE STT per
    chunk; tanh(C) = tanh(C' * 0.5) uses ACT's free scale; the peephole
    scalars are halved on the host (p * C = (p/2) * C').
  - DMA rings: sync carries the small L1 pack [36, 466] and w2k1;
    gpsimd SWDGE (idle Pool engine) carries the big L2+FC pack - an
    InstDMACopy on the ACT queue made the act-table pass emit a second
    1.3us table load in v3, so the ACT queue now has activations only.
  - The O-gate sigmoids are whole-width (no chunk split); the garbage
    partition rows they read are zero-memset early on DVE.
  - Output DMA is split across the sync+scalar rings so the two ~0.5us
    HWDGE issue windows overlap.

Sharding: problem is tiny (N=35); all 8 cores run the identical program
on replicated inputs (no collectives), output taken from core 0.
"""

import sys

for _p in ("/opt/trn_rl_repo", "/opt/pypackages"):
    if _p not in sys.path:
        sys.path.append(_p)

from contextlib import ExitStack

import numpy as np
import ml_dtypes

import concourse.bacc as bacc
import concourse.bass as bass
import concourse.mybir as mybir
import concourse.tile as tile

F32 = mybir.dt.float32
BF16 = mybir.dt.bfloat16
AF = mybir.ActivationFunctionType
OP = mybir.AluOpType
GATES = (0, 2, 3)  # I, T (cell), O - forget gate (1) is dead
N = 35
D1 = 140
D2 = 280
N_CORES = 8
BF = ml_dtypes.bfloat16

# L1 feature chunks (140 = 128 + 12), L2 chunks (280 = 128 + 256 + 24).
C1 = ((0, 128), (128, 140))
C2 = ((0, 128), (128, 256), (256, 280))


def build_nc() -> bass.Bass:
    nc = bacc.Bacc()

    # wpk: [36, 466] bf16 - adj + ALL layer-1 weights in one transfer:
    #   cols 0:36 = [adj; ones], 36:176 = [W1[I]/2; bias/2],
    #   176:316 = [W1[T]; bias], 316:456 = [W1[O]; bias], 456:466 pad.
    wpk = nc.dram_tensor("wpk", [36, 466], BF16, kind="ExternalInput")
    # W2 k-chunk1 (input features 128:140 + bias row): [13, 3, 280],
    # gate order (I/2, T, O).
    w2k1 = nc.dram_tensor("w2k1", [13, 3, D2], BF16, kind="ExternalInput")
    # wb: [128, 958] bf16 - layer-2 + FC weights (k-chunk0) + peepholes:
    #   cols 0:280 = W2[I]/2, 280:560 = W2[T], 560:840 = W2[O],
    #   840:948 = [fc_w; fc_b] row-chunks as 3x36, 948:958 = halved
    #   peep1[2]/peep2[2] scalars as a bit-cast fp32 [128, 5] block.
    wb = nc.dram_tensor("wb", [128, 958], BF16, kind="ExternalInput")
    out = nc.dram_tensor("out", [N, N], F32, kind="ExternalOutput")

    out_sb = nc.alloc_sbuf_tensor("out_sbuf", [N, N], F32)
    out_sem = nc.alloc_semaphore("out_dma_sem")

    with ExitStack() as ctx:
        tc = ctx.enter_context(tile.TileContext(nc))
        sb = ctx.enter_context(tc.tile_pool(name="sb", bufs=1))
        ps1 = ctx.enter_context(tc.tile_pool(name="ps1", bufs=2, space="PSUM"))
        ps2 = ctx.enter_context(tc.tile_pool(name="ps2", bufs=3, space="PSUM"))

        # ---- input DMAs: three independent queues issue in parallel ----
        wpk_sb = sb.tile([36, 466], BF16, tag="wpk")
        nc.sync.dma_start(out=wpk_sb, in_=wpk[:, :])
        wb_sb = sb.tile([128, 958], BF16, tag="wb")
        nc.gpsimd.dma_start(out=wb_sb, in_=wb[:, :])
        w2k1_sb = sb.tile([13, 3, D2], BF16, tag="w2k1")
        nc.sync.dma_start(out=w2k1_sb, in_=w2k1[:, :, :])

        adjp = wpk_sb[0:36, 0:36]
        aux_v = wb_sb[:, 948:958].bitcast(F32)  # [128, 5] peep/2 scalars

        # ones rows for the bias folds + zero-fill of the garbage rows
        # that the whole-width O sigmoids / tanhs read.
        x1T = sb.tile([128, 72], BF16, tag="x1T")
        nc.vector.memset(x1T[0:13, 36:72], 1.0)
        x2T = sb.tile([128, 108], BF16, tag="x2T")
        nc.vector.memset(x2T[0:25, 72:108], 1.0)
        cp1 = sb.tile([128, 72], BF16, tag="cp1")    # C' = 2C, layer 1
        nc.vector.memset(cp1[0:128, 36:72], 0.0)
        po1 = sb.tile([128, 72], BF16, tag="po1")
        nc.vector.memset(po1[0:128, 36:72], 0.0)
        cp2 = sb.tile([128, 108], BF16, tag="cp2")   # C' = 2C, layer 2
        nc.vector.memset(cp2[0:128, 72:108], 0.0)
        po2 = sb.tile([128, 108], BF16, tag="po2")
        nc.vector.memset(po2[0:128, 72:108], 0.0)

        # ---- layer 1 (transposed): I+T interleaved in one psum bank ----
        # p1it cols: [Ic0 | Tc0 | Ic1 | Tc1] (36 each); p1o: [Oc0 | Oc1].
        p1it = ps1.tile([128, 144], F32, tag="ps1", name="p1it")
        p1o = ps1.tile([128, 72], F32, tag="ps1", name="p1o")
        l1_lhs = (
            lambda a, b: wpk_sb[0:36, 36 + a : 36 + b],   # I/2
            lambda a, b: wpk_sb[0:36, 176 + a : 176 + b],  # T
            lambda a, b: wpk_sb[0:36, 316 + a : 316 + b],  # O
        )
        for ci, (a, b) in enumerate(C1):  # regions Ic0,Tc0 then Ic1,Tc1
            for g in (0, 1):
                nc.tensor.matmul(
                    p1it[0 : b - a, (2 * ci + g) * 36 : (2 * ci + g) * 36 + 36],
                    lhsT=l1_lhs[g](a, b),
                    rhs=adjp,
                    start=True,
                    stop=True,
                )
        for ci, (a, b) in enumerate(C1):
            nc.tensor.matmul(
                p1o[0 : b - a, ci * 36 : ci * 36 + 36],
                lhsT=l1_lhs[2](a, b),
                rhs=adjp,
                start=True,
                stop=True,
            )
        # t = tanh([aI/2 | aT]) per chunk; C' = (1+tI)*tT per chunk
        t1 = sb.tile([128, 144], BF16, tag="t1")
        nc.scalar.activation(t1[0:128, 0:72], p1it[0:128, 0:72], AF.Tanh)
        nc.scalar.activation(t1[0:12, 72:144], p1it[0:12, 72:144], AF.Tanh)
        nc.vector.scalar_tensor_tensor(
            cp1[0:128, 0:36], in0=t1[0:128, 0:36], scalar=1.0,
            in1=t1[0:128, 36:72], op0=OP.add, op1=OP.mult,
        )
        nc.vector.scalar_tensor_tensor(
            cp1[0:12, 36:72], in0=t1[0:12, 72:108], scalar=1.0,
            in1=t1[0:12, 108:144], op0=OP.add, op1=OP.mult,
        )
        # po = (p/2)*C' + aO per chunk; tanh(C) = tanh(C'/2) whole-width
        for ci, (a, b) in enumerate(C1):
            cs = b - a
            nc.vector.scalar_tensor_tensor(
                po1[0:cs, ci * 36 : ci * 36 + 36],
                in0=cp1[0:cs, ci * 36 : ci * 36 + 36],
                scalar=aux_v[0:cs, ci : ci + 1],
                in1=p1o[0:cs, ci * 36 : ci * 36 + 36],
                op0=OP.mult, op1=OP.add,
            )
        tc1 = sb.tile([128, 72], BF16, tag="tc1")
        nc.scalar.activation(tc1, cp1[:, :], AF.Tanh, scale=0.5)
        go1 = sb.tile([128, 72], BF16, tag="go1")
        nc.scalar.activation(go1, po1[:, :], AF.Sigmoid)
        # x1T = max(tanh(C), 0) * O
        nc.vector.scalar_tensor_tensor(
            x1T[0:128, 0:36], in0=tc1[0:128, 0:36], scalar=0.0,
            in1=go1[0:128, 0:36], op0=OP.max, op1=OP.mult,
        )
        nc.vector.scalar_tensor_tensor(
            x1T[0:12, 36:72], in0=tc1[0:12, 36:72], scalar=0.0,
            in1=go1[0:12, 36:72], op0=OP.max, op1=OP.mult,
        )

        # ---- layer 2: I+T interleaved [Ic0|Tc0|Ic1|Tc1|Ic2|Tc2] ----
        p2it = ps2.tile([128, 216], F32, tag="ps2", name="p2it")
        p2o = ps2.tile([128, 108], F32, tag="ps2", name="p2o")
        w2k0 = (wb_sb[:, 0:280], wb_sb[:, 280:560], wb_sb[:, 560:840])
        for ci, (a, b) in enumerate(C2):
            for g in (0, 1):
                col = (2 * ci + g) * 36
                nc.tensor.matmul(
                    p2it[0 : b - a, col : col + 36],
                    lhsT=w2k0[g][:, a:b],
                    rhs=x1T[0:128, 0:36],
                    start=True, stop=False,
                )
                nc.tensor.matmul(
                    p2it[0 : b - a, col : col + 36],
                    lhsT=w2k1_sb[0:13, g, a:b],
                    rhs=x1T[0:13, 36:72],
                    start=False, stop=True,
                )
        for ci, (a, b) in enumerate(C2):
            col = ci * 36
            nc.tensor.matmul(
                p2o[0 : b - a, col : col + 36],
                lhsT=w2k0[2][:, a:b],
                rhs=x1T[0:128, 0:36],
                start=True, stop=False,
            )
            nc.tensor.matmul(
                p2o[0 : b - a, col : col + 36],
                lhsT=w2k1_sb[0:13, 2, a:b],
                rhs=x1T[0:13, 36:72],
                start=False, stop=True,
            )
        t2 = sb.tile([128, 216], BF16, tag="t2")
        nc.scalar.activation(t2[0:128, 0:72], p2it[0:128, 0:72], AF.Tanh)
        nc.scalar.activation(t2[0:128, 72:144], p2it[0:128, 72:144], AF.Tanh)
        nc.scalar.activation(t2[0:24, 144:216], p2it[0:24, 144:216], AF.Tanh)
        cp2_pieces = ((0, 128, 0), (1, 128, 36), (2, 24, 72))
        for ci, cs, col in cp2_pieces:
            nc.vector.scalar_tensor_tensor(
                cp2[0:cs, col : col + 36],
                in0=t2[0:cs, 2 * ci * 36 : 2 * ci * 36 + 36], scalar=1.0,
                in1=t2[0:cs, (2 * ci + 1) * 36 : (2 * ci + 1) * 36 + 36],
                op0=OP.add, op1=OP.mult,
            )
        for ci, (a, b) in enumerate(C2):
            cs = b - a
            nc.vector.scalar_tensor_tensor(
                po2[0:cs, ci * 36 : ci * 36 + 36],
                in0=cp2[0:cs, ci * 36 : ci * 36 + 36],
                scalar=aux_v[0:cs, 2 + ci : 3 + ci],
                in1=p2o[0:cs, ci * 36 : ci * 36 + 36],
                op0=OP.mult, op1=OP.add,
            )
        tc2 = sb.tile([128, 108], BF16, tag="tc2")
        nc.scalar.activation(tc2, cp2[:, :], AF.Tanh, scale=0.5)
        go2 = sb.tile([128, 108], BF16, tag="go2")
        nc.scalar.activation(go2, po2[:, :], AF.Sigmoid)
        nc.vector.scalar_tensor_tensor(
            x2T[0:128, 0:72], in0=tc2[0:128, 0:72], scalar=0.0,
            in1=go2[0:128, 0:72], op0=OP.max, op1=OP.mult,
        )
        nc.vector.scalar_tensor_tensor(
            x2T[0:24, 72:108], in0=tc2[0:24, 72:108], scalar=0.0,
            in1=go2[0:24, 72:108], op0=OP.max, op1=OP.mult,
        )
        psfc = ps2.tile([N, 36], F32, tag="ps2", name="psfc")
        nc.tensor.matmul(
            psfc, lhsT=x2T[0:128, 0:35], rhs=wb_sb[:, 840:876],
            start=True, stop=False,
        )
        nc.tensor.matmul(
            psfc, lhsT=x2T[0:128, 36:71], rhs=wb_sb[:, 876:912],
            start=False, stop=False,
        )
        nc.tensor.matmul(
            psfc, lhsT=x2T[0:25, 72:107], rhs=wb_sb[0:25, 912:948],
            start=False, stop=True,
        )
        nc.vector.tensor_scalar_max(out_sb[0:N, 0:N], psfc[:, 0:N], 0.0)

    # Output DMA split across the two HWDGE rings so the issue windows
    # overlap; both fire-and-forget under the compiler's sem-reset
    # epilogue (ordered by the context-exit all-engine barrier).
    nc.sync.dma_start(out=out[0:18, :], in_=out_sb[0:18, 0:N]).then_inc(out_sem, 16)
    nc.scalar.dma_start(out=out[18:35, :], in_=out_sb[18:35, 0:N]).then_inc(
        out_sem, 16
    )

    nc.compile()
    return nc


def pack_inputs(
    adj_matrix, W1, cheb1_b, peep1, b1, W2, cheb2_b, peep2, b2, fc_w, fc_b
) -> dict:
    """Host-side weight packing: gather/concat + bias fold + bf16 cast.

    The I-gate weights (and biases) of both layers and the peephole
    scalars are halved: the kernel computes I via tanh(aI/2)."""
    f = np.float32

    def gate_blk(Wg, bias, scale=1.0):  # [k+1, d] with the bias fold row
        blk = np.concatenate([Wg, bias[None, :]], axis=0) * scale
        return blk.astype(BF)

    adjp = np.zeros((36, 36), dtype=f)
    adjp[0:35, 0:35] = adj_matrix
    adjp[35, 0:35] = 1.0

    wpk_h = np.zeros((36, 466), dtype=BF)
    wpk_h[:, 0:36] = adjp.astype(BF)
    wpk_h[:, 36:176] = gate_blk(W1[0], cheb1_b[0] + b1[0], 0.5)
    wpk_h[:, 176:316] = gate_blk(W1[2], cheb1_b[2] + b1[2])
    wpk_h[:, 316:456] = gate_blk(W1[3], cheb1_b[3] + b1[3])

    scales = {0: 0.5, 2: 1.0, 3: 1.0}
    w2k1_h = np.stack(
        [gate_blk(W2[g][128:140], cheb2_b[g] + b2[g], scales[g]) for g in GATES],
        axis=1,
    )  # [13, 3, 280]

    wb_h = np.zeros((128, 958), dtype=BF)
    wb_h[:, 0:280] = (W2[0][0:128] * 0.5).astype(BF)
    wb_h[:, 280:560] = W2[2][0:128].astype(BF)
    wb_h[:, 560:840] = W2[3][0:128].astype(BF)
    fcx = np.concatenate([fc_w, fc_b[None, :]], axis=0)  # [281, 35]
    wb_h[:, 840:875] = fcx[0:128].astype(BF)
    wb_h[:, 876:911] = fcx[128:256].astype(BF)
    wb_h[0:25, 912:947] = fcx[256:281].astype(BF)
    aux_h = np.zeros((128, 5), dtype=f)
    aux_h[:, 0] = peep1[2][0:128] * 0.5
    aux_h[0:12, 1] = peep1[2][128:140] * 0.5
    aux_h[:, 2] = peep2[2][0:128] * 0.5
    aux_h[:, 3] = peep2[2][128:256] * 0.5
    aux_h[0:24, 4] = peep2[2][256:280] * 0.5
    wb_h[:, 948:958] = np.ascontiguousarray(aux_h).view(BF)

    return {
        "wpk": np.ascontiguousarray(wpk_h),
        "w2k1": np.ascontiguousarray(w2k1_h),
        "wb": np.ascontiguousarray(wb_h),
    }


_NC_CACHE: list = []


def kernel(
    adj_matrix,
    W1,
    cheb1_W,
    cheb1_b,
    peep1,
    b1,
    W2,
    cheb2_W,
    cheb2_b,
    peep2,
    b2,
    fc_w,
    fc_b,
) -> np.ndarray:
    from concourse.bass_utils import run_bass_kernel_spmd

    in_map = pack_inputs(
        adj_matrix, W1, cheb1_b, peep1, b1, W2, cheb2_b, peep2, b2, fc_w, fc_b
    )

    if not _NC_CACHE:
        _NC_CACHE.append(build_nc())
    nc = _NC_CACHE[0]

    in_maps = [dict(in_map) for _ in range(N_CORES)]
    try:
        res = run_bass_kernel_spmd(nc, in_maps, core_ids=list(range(N_CORES)))
    except Exception:
        # transient device wedges (NRT_EXEC_*) usually clear on re-run
        res = run_bass_kernel_spmd(nc, in_maps, core_ids=list(range(N_CORES)))
    return np.asarray(res.results[0]["out"], dtype=np.float32)


# revision 12
# speedup vs baseline: 1.0281x; 1.0281x over previous
"""Trainium2 Bass kernel for nn_GCLSTMModel_48868137894020 (v4).

Algebraic reduction (H0 = C0 = 0 kills the cheb convs, the forget gate,
and peep[0/1]); per layer (d = 140 then 280), X = input:
  I = sigmoid(X @ W[0] + cb[0] + b[0])
  T = tanh   (X @ W[2] + cb[2] + b[2])
  C = I * T
  O = sigmoid(X @ W[3] + cb[3] + b[3] + peep[2] * C)
  X' = relu(O * tanh(C)) == max(tanh(C), 0) * O
then out = relu(X'' @ fc_w + fc_b).

v4 on top of v3's two-ring DMA plan:
  - sigmoid(x) = (1 + tanh(x/2)) / 2: the I-gate weights are halved on
    the host, and I+T land interleaved per-chunk in ONE psum bank, so
    one tanh ACTIVATE per chunk covers both gates (the ACT engine was
    the tail bottleneck).  C' := 2C = (1 + tI) * tT is one DVE STT per
    chunk; tanh(C) = tanh(C' * 0.5) uses ACT's free scale; the peephole
    scalars are halved on the host (p * C = (p/2) * C').
  - DMA rings: sync carries the L1 pack [128, 466] then the L2+FC pack
    [128, 948]; gpsimd SWDGE (idle Pool engine) carries the small w2k1.
    No DMA rides the ACT queue: that made the act-table pass emit a
    spurious extra 1.3us table load in v3.
  - A single warm-up SIGMOID heads the ACT queue: the act-table pass
    greedily picks the table set for the FIRST activation function, and
    the sigmoid set contains tanh too (the reverse is not true - v4 ate
    a 1.3us mid-chain table load by leading with tanh).
  - The O-gate matmul regions are interleaved per chunk ([I,T,O] x c)
    so the peephole STTs run while later chunks are still streaming.

Sharding: problem is tiny (N=35); all 8 cores run the identical program
on replicated inputs (no collectives), output taken from core 0.
"""

import sys

for _p in ("/opt/trn_rl_repo", "/opt/pypackages"):
    if _p not in sys.path:
        sys.path.append(_p)

from contextlib import ExitStack

import numpy as np
import ml_dtypes

import concourse.bacc as bacc
import concourse.bass as bass
import concourse.mybir as mybir
import concourse.tile as tile

F32 = mybir.dt.float32
BF16 = mybir.dt.bfloat16
AF = mybir.ActivationFunctionType
OP = mybir.AluOpType
GATES = (0, 2, 3)  # I, T (cell), O - forget gate (1) is dead
N = 35
D1 = 140
D2 = 280
N_CORES = 8
BF = ml_dtypes.bfloat16

# L1 feature chunks (140 = 128 + 12), L2 chunks (280 = 128 + 256 + 24).
C1 = ((0, 128), (128, 140))
C2 = ((0, 128), (128, 256), (256, 280))


def build_nc() -> bass.Bass:
    nc = bacc.Bacc()

    # wpk: [128, 466] bf16 - adj + ALL layer-1 weights + peepholes:
    #   rows 0:36: cols 0:36 = [adj; ones], 36:176 = [W1[I]/2; bias/2],
    #   176:316 = [W1[T]; bias], 316:456 = [W1[O]; bias]; cols 456:466
    #   rows 0:128 = halved peep1[2]/peep2[2] scalars as a bit-cast
    #   fp32 [128, 5] block.
    wpk = nc.dram_tensor("wpk", [128, 466], BF16, kind="ExternalInput")
    # W2 k-chunk1 (input features 128:140 + bias row): [13, 3, 280],
    # gate order (I/2, T, O).
    w2k1 = nc.dram_tensor("w2k1", [13, 3, D2], BF16, kind="ExternalInput")
    # wb: [128, 948] bf16 - layer-2 + FC weights (k-chunk0):
    #   cols 0:280 = W2[I]/2, 280:560 = W2[T], 560:840 = W2[O],
    #   840:948 = [fc_w; fc_b] row-chunks as 3x36.
    wb = nc.dram_tensor("wb", [128, 948], BF16, kind="ExternalInput")
    out = nc.dram_tensor("out", [N, N], F32, kind="ExternalOutput")

    out_sb = nc.alloc_sbuf_tensor("out_sbuf", [N, N], F32)
    out_sem = nc.alloc_semaphore("out_dma_sem")

    with ExitStack() as ctx:
        tc = ctx.enter_context(tile.TileContext(nc))
        sb = ctx.enter_context(tc.tile_pool(name="sb", bufs=1))
        ps1 = ctx.enter_context(tc.tile_pool(name="ps1", bufs=2, space="PSUM"))
        ps2 = ctx.enter_context(tc.tile_pool(name="ps2", bufs=3, space="PSUM"))

        # ---- input DMAs: sync ring + SWDGE issue in parallel ----
        wpk_sb = sb.tile([128, 466], BF16, tag="wpk")
        nc.sync.dma_start(out=wpk_sb, in_=wpk[:, :])
        wb_sb = sb.tile([128, 948], BF16, tag="wb")
        nc.sync.dma_start(out=wb_sb, in_=wb[:, :])
        w2k1_sb = sb.tile([13, 3, D2], BF16, tag="w2k1")
        nc.gpsimd.dma_start(out=w2k1_sb, in_=w2k1[:, :, :])

        adjp = wpk_sb[0:36, 0:36]
        aux_v = wpk_sb[:, 456:466].bitcast(F32)  # [128, 5] peep/2 scalars

        # warm-up sigmoid with no DMA deps: the act-table pass keys the
        # table set off the first activation; the sigmoid set covers
        # tanh too, so this pins ONE table load into the DMA shadow.
        warm_src = sb.tile([1, 1], F32, tag="warm_src")
        nc.vector.memset(warm_src[:, :], 0.25)
        warm = sb.tile([1, 1], F32, tag="warm")
        nc.scalar.activation(warm[0:1, 0:1], warm_src[0:1, 0:1], AF.Sigmoid)

        # ones rows for the bias folds + zero-fill of the garbage rows
        # that the whole-width O sigmoids / tanhs read.
        x1T = sb.tile([128, 72], BF16, tag="x1T")
        nc.vector.memset(x1T[0:13, 36:72], 1.0)
        x2T = sb.tile([128, 108], BF16, tag="x2T")
        nc.vector.memset(x2T[0:25, 72:108], 1.0)
        cp1 = sb.tile([128, 72], BF16, tag="cp1")    # C' = 2C, layer 1
        nc.vector.memset(cp1[0:128, 36:72], 0.0)
        po1 = sb.tile([128, 72], BF16, tag="po1")
        nc.vector.memset(po1[0:128, 36:72], 0.0)
        cp2 = sb.tile([128, 108], BF16, tag="cp2")   # C' = 2C, layer 2
        nc.vector.memset(cp2[0:128, 72:108], 0.0)
        po2 = sb.tile([128, 108], BF16, tag="po2")
        nc.vector.memset(po2[0:128, 72:108], 0.0)

        # ---- layer 1 (transposed): I+T interleaved in one psum bank ----
        # p1it cols: [Ic0 | Tc0 | Ic1 | Tc1] (36 each); p1o: [Oc0 | Oc1].
        p1it = ps1.tile([128, 144], F32, tag="ps1", name="p1it")
        p1o = ps1.tile([128, 72], F32, tag="ps1", name="p1o")
        l1_lhs = (
            lambda a, b: wpk_sb[0:36, 36 + a : 36 + b],   # I/2
            lambda a, b: wpk_sb[0:36, 176 + a : 176 + b],  # T
            lambda a, b: wpk_sb[0:36, 316 + a : 316 + b],  # O
        )
        for ci, (a, b) in enumerate(C1):  # per chunk: I, T then O
            for g in (0, 1):
                nc.tensor.matmul(
                    p1it[0 : b - a, (2 * ci + g) * 36 : (2 * ci + g) * 36 + 36],
                    lhsT=l1_lhs[g](a, b),
                    rhs=adjp,
                    start=True,
                    stop=True,
                )
            nc.tensor.matmul(
                p1o[0 : b - a, ci * 36 : ci * 36 + 36],
                lhsT=l1_lhs[2](a, b),
                rhs=adjp,
                start=True,
                stop=True,
            )
        # t = tanh([aI/2 | aT]) per chunk; C' = (1+tI)*tT per chunk
        t1 = sb.tile([128, 144], BF16, tag="t1")
        nc.scalar.activation(t1[0:128, 0:72], p1it[0:128, 0:72], AF.Tanh)
        nc.scalar.activation(t1[0:12, 72:144], p1it[0:12, 72:144], AF.Tanh)
        # DVE per chunk: C' = (1+tI)*tT, then po = (p/2)*C' + aO - the
        # chunk-0 pair is emitted before any chunk-1 dependency so a
        # stalled cp1-c1 can't head-block the DVE queue.
        cp1_srcs = ((128, 0, 36, 72), (12, 36, 108, 144))
        for ci, (cs, col, s0, s1) in enumerate(cp1_srcs):
            nc.vector.scalar_tensor_tensor(
                cp1[0:cs, col : col + 36], in0=t1[0:cs, s0 - 36 : s0],
                scalar=1.0, in1=t1[0:cs, s1 - 36 : s1],
                op0=OP.add, op1=OP.mult,
            )
            nc.vector.scalar_tensor_tensor(
                po1[0:cs, col : col + 36],
                in0=cp1[0:cs, col : col + 36],
                scalar=aux_v[0:cs, ci : ci + 1],
                in1=p1o[0:cs, col : col + 36],
                op0=OP.mult, op1=OP.add,
            )
        tc1 = sb.tile([128, 72], BF16, tag="tc1")
        nc.scalar.activation(tc1, cp1[:, :], AF.Tanh, scale=0.5)
        go1 = sb.tile([128, 72], BF16, tag="go1")
        nc.scalar.activation(go1, po1[:, :], AF.Sigmoid)
        # x1T = max(tanh(C), 0) * O
        nc.vector.scalar_tensor_tensor(
            x1T[0:128, 0:36], in0=tc1[0:128, 0:36], scalar=0.0,
            in1=go1[0:128, 0:36], op0=OP.max, op1=OP.mult,
        )
        nc.vector.scalar_tensor_tensor(
            x1T[0:12, 36:72], in0=tc1[0:12, 36:72], scalar=0.0,
            in1=go1[0:12, 36:72], op0=OP.max, op1=OP.mult,
        )

        # ---- layer 2: I+T interleaved [Ic0|Tc0|Ic1|Tc1|Ic2|Tc2] ----
        p2it = ps2.tile([128, 216], F32, tag="ps2", name="p2it")
        p2o = ps2.tile([128, 108], F32, tag="ps2", name="p2o")
        w2k0 = (wb_sb[:, 0:280], wb_sb[:, 280:560], wb_sb[:, 560:840])
        for ci, (a, b) in enumerate(C2):  # per chunk: I, T then O
            for g, dst, col in (
                (0, p2it, 2 * ci * 36),
                (1, p2it, (2 * ci + 1) * 36),
                (2, p2o, ci * 36),
            ):
                nc.tensor.matmul(
                    dst[0 : b - a, col : col + 36],
                    lhsT=w2k0[g][:, a:b],
                    rhs=x1T[0:128, 0:36],
                    start=True, stop=False,
                )
                nc.tensor.matmul(
                    dst[0 : b - a, col : col + 36],
                    lhsT=w2k1_sb[0:13, g, a:b],
                    rhs=x1T[0:13, 36:72],
                    start=False, stop=True,
                )
        t2 = sb.tile([128, 216], BF16, tag="t2")
        nc.scalar.activation(t2[0:128, 0:72], p2it[0:128, 0:72], AF.Tanh)
        nc.scalar.activation(t2[0:128, 72:144], p2it[0:128, 72:144], AF.Tanh)
        nc.scalar.activation(t2[0:24, 144:216], p2it[0:24, 144:216], AF.Tanh)
        for ci, cs in ((0, 128), (1, 128), (2, 24)):
            col = ci * 36
            nc.vector.scalar_tensor_tensor(
                cp2[0:cs, col : col + 36],
                in0=t2[0:cs, 2 * col : 2 * col + 36], scalar=1.0,
                in1=t2[0:cs, 2 * col + 36 : 2 * col + 72],
                op0=OP.add, op1=OP.mult,
            )
            nc.vector.scalar_tensor_tensor(
                po2[0:cs, col : col + 36],
                in0=cp2[0:cs, col : col + 36],
                scalar=aux_v[0:cs, 2 + ci : 3 + ci],
                in1=p2o[0:cs, col : col + 36],
                op0=OP.mult, op1=OP.add,
            )
        tc2 = sb.tile([128, 108], BF16, tag="tc2")
        nc.scalar.activation(tc2, cp2[:, :], AF.Tanh, scale=0.5)
        go2 = sb.tile([128, 108], BF16, tag="go2")
        nc.scalar.activation(go2, po2[:, :], AF.Sigmoid)
        nc.vector.scalar_tensor_tensor(
            x2T[0:128, 0:72], in0=tc2[0:128, 0:72], scalar=0.0,
            in1=go2[0:128, 0:72], op0=OP.max, op1=OP.mult,
        )
        nc.vector.scalar_tensor_tensor(
            x2T[0:24, 72:108], in0=tc2[0:24, 72:108], scalar=0.0,
            in1=go2[0:24, 72:108], op0=OP.max, op1=OP.mult,
        )
        psfc = ps2.tile([N, 36], F32, tag="ps2", name="psfc")
        nc.tensor.matmul(
            psfc, lhsT=x2T[0:128, 0:35], rhs=wb_sb[:, 840:876],
            start=True, stop=False,
        )
        nc.tensor.matmul(
            psfc, lhsT=x2T[0:128, 36:71], rhs=wb_sb[:, 876:912],
            start=False, stop=False,
        )
        nc.tensor.matmul(
            psfc, lhsT=x2T[0:25, 72:107], rhs=wb_sb[0:25, 912:948],
            start=False, stop=True,
        )
        nc.vector.tensor_scalar_max(out_sb[0:N, 0:N], psfc[:, 0:N], 0.0)

    # Output DMA after the tile context, on the warm sync ring (the
    # scalar ring is cold here: first issue is ~2x as long).  Fire and
    # forget: its flight hides under the compiler's fixed sem-reset
    # epilogue; ordering comes from the context-exit barrier.
    nc.sync.dma_start(out=out[:, :], in_=out_sb[0:N, 0:N]).then_inc(out_sem, 16)

    nc.compile()
    return nc


def pack_inputs(
    adj_matrix, W1, cheb1_b, peep1, b1, W2, cheb2_b, peep2, b2, fc_w, fc_b
) -> dict:
    """Host-side weight packing: gather/concat + bias fold + bf16 cast.

    The I-gate weights (and biases) of both layers and the peephole
    scalars are halved: the kernel computes I via tanh(aI/2)."""
    f = np.float32

    def gate_blk(Wg, bias, scale=1.0):  # [k+1, d] with the bias fold row
        blk = np.concatenate([Wg, bias[None, :]], axis=0) * scale
        return blk.astype(BF)

    adjp = np.zeros((36, 36), dtype=f)
    adjp[0:35, 0:35] = adj_matrix
    adjp[35, 0:35] = 1.0

    wpk_h = np.zeros((128, 466), dtype=BF)
    wpk_h[0:36, 0:36] = adjp.astype(BF)
    wpk_h[0:36, 36:176] = gate_blk(W1[0], cheb1_b[0] + b1[0], 0.5)
    wpk_h[0:36, 176:316] = gate_blk(W1[2], cheb1_b[2] + b1[2])
    wpk_h[0:36, 316:456] = gate_blk(W1[3], cheb1_b[3] + b1[3])
    aux_h = np.zeros((128, 5), dtype=f)
    aux_h[:, 0] = peep1[2][0:128] * 0.5
    aux_h[0:12, 1] = peep1[2][128:140] * 0.5
    aux_h[:, 2] = peep2[2][0:128] * 0.5
    aux_h[:, 3] = peep2[2][128:256] * 0.5
    aux_h[0:24, 4] = peep2[2][256:280] * 0.5
    wpk_h[:, 456:466] = np.ascontiguousarray(aux_h).view(BF)

    scales = {0: 0.5, 2: 1.0, 3: 1.0}
    w2k1_h = np.stack(
        [gate_blk(W2[g][128:140], cheb2_b[g] + b2[g], scales[g]) for g in GATES],
        axis=1,
    )  # [13, 3, 280]

    wb_h = np.zeros((128, 948), dtype=BF)
    wb_h[:, 0:280] = (W2[0][0:128] * 0.5).astype(BF)
    wb_h[:, 280:560] = W2[2][0:128].astype(BF)
    wb_h[:, 560:840] = W2[3][0:128].astype(BF)
    fcx = np.concatenate([fc_w, fc_b[None, :]], axis=0)  # [281, 35]
    wb_h[:, 840:875] = fcx[0:128].astype(BF)
    wb_h[:, 876:911] = fcx[128:256].astype(BF)
    wb_h[0:25, 912:947] = fcx[256:281].astype(BF)

    return {
        "wpk": np.ascontiguousarray(wpk_h),
        "w2k1": np.ascontiguousarray(w2k1_h),
        "wb": np.ascontiguousarray(wb_h),
    }


_NC_CACHE: list = []


def kernel(
    adj_matrix,
    W1,
    cheb1_W,
    cheb1_b,
    peep1,
    b1,
    W2,
    cheb2_W,
    cheb2_b,
    peep2,
    b2,
    fc_w,
    fc_b,
) -> np.ndarray:
    from concourse.bass_utils import run_bass_kernel_spmd

    in_map = pack_inputs(
        adj_matrix, W1, cheb1_b, peep1, b1, W2, cheb2_b, peep2, b2, fc_w, fc_b
    )

    if not _NC_CACHE:
        _NC_CACHE.append(build_nc())
    nc = _NC_CACHE[0]

    in_maps = [dict(in_map) for _ in range(N_CORES)]
    try:
        res = run_bass_kernel_spmd(nc, in_maps, core_ids=list(range(N_CORES)))
    except Exception:
        # transient device wedges (NRT_EXEC_*) usually clear on re-run
        res = run_bass_kernel_spmd(nc, in_maps, core_ids=list(range(N_CORES)))
    return np.asarray(res.results[0]["out"], dtype=np.float32)


# revision 19
# speedup vs baseline: 1.0690x; 1.0398x over previous
"""Trainium2 Bass kernel for nn_GCLSTMModel_48868137894020 (v6).

Algebraic reduction (H0 = C0 = 0 kills the cheb convs, the forget gate,
and peep[0/1]); per layer (d = 140 then 280), X = input:
  I = sigmoid(X @ W[0] + cb[0] + b[0])
  T = tanh   (X @ W[2] + cb[2] + b[2])
  C = I * T
  O = sigmoid(X @ W[3] + cb[3] + b[3] + peep[2] * C)
  X' = relu(O * tanh(C)) == max(tanh(C), 0) * O
then out = relu(X'' @ fc_w + fc_b).

Layout: everything transposed (features on partitions, nodes on the
free dim), bf16 matmuls with fp32 PSUM, biases folded via a ones row.

v6 specifics (informed by the v3/v4/v5 traces):
  - sigmoid(x) = (1 + tanh(x/2)) / 2: I-gate weights halved on the
    host, so ONE tanh ACTIVATE per chunk covers gates I and T together
    (ACT-engine serialization was the tail bottleneck).  C' := 2C =
    (1 + tI) * tT is one DVE STT per chunk; tanh(C) = tanh(0.5 * C')
    uses ACT's free scale; peephole scalars are halved on the host.
  - Tile tracks PSUM dependencies at whole-tile granularity, so every
    matmul REGION gets its own psum tile (p1a/p1b/p1o0/... ) - readers
    then wait only for their own producers.
  - All three weight DMAs ride the sync HWDGE ring in need-order
    (L1 pack, w2k1, L2+FC pack): a DMA on the ACT queue triggers a
    spurious act-table load (v3), and SWDGE adds a multi-us drain (v5).
  - A single warm-up SIGMOID heads the ACT queue: the act-table pass
    keys the (one) table load off the first activation function, and
    the sigmoid set also contains tanh - leading with tanh loads a
    tanh-only set and re-loads mid-chain (v4, +1.3us).
  - Output DMA on the warm sync ring after the tile context; its
    flight hides under the compiler's fixed sem-reset epilogue.

Sharding: problem is tiny (N=35); all 8 cores run the identical program
on replicated inputs (no collectives), output taken from core 0.
"""

import sys

for _p in ("/opt/trn_rl_repo", "/opt/pypackages"):
    if _p not in sys.path:
        sys.path.append(_p)

from contextlib import ExitStack

import numpy as np
import ml_dtypes

import concourse.bacc as bacc
import concourse.bass as bass
import concourse.mybir as mybir
import concourse.tile as tile

F32 = mybir.dt.float32
BF16 = mybir.dt.bfloat16
AF = mybir.ActivationFunctionType
OP = mybir.AluOpType
GATES = (0, 2, 3)  # I, T (cell), O - forget gate (1) is dead
N = 35
D1 = 140
D2 = 280
N_CORES = 8
BF = ml_dtypes.bfloat16

# L1 feature chunks (140 = 128 + 12), L2 chunks (280 = 128 + 256 + 24).
C1 = ((0, 128), (128, 140))
C2 = ((0, 128), (128, 256), (256, 280))


def build_nc() -> bass.Bass:
    nc = bacc.Bacc()

    # wpk: [128, 466] bf16 - adj + ALL layer-1 weights + peepholes:
    #   rows 0:36: cols 0:36 = [adj; ones], 36:176 = [W1[I]/2; bias/2],
    #   176:316 = [W1[T]; bias], 316:456 = [W1[O]; bias]; cols 456:466
    #   rows 0:128 = halved peep scalars as a bit-cast fp32 [128, 5].
    wpk = nc.dram_tensor("wpk", [128, 466], BF16, kind="ExternalInput")
    # W2 k-chunk1 (input features 128:140 + bias row): [13, 3, 280],
    # gate order (I/2, T, O).
    w2k1 = nc.dram_tensor("w2k1", [13, 3, D2], BF16, kind="ExternalInput")
    # wb: [128, 948] bf16 - layer-2 + FC weights (k-chunk0):
    #   cols 0:280 = W2[I]/2, 280:560 = W2[T], 560:840 = W2[O],
    #   840:948 = [fc_w; fc_b] row-chunks as 3x36.
    wb = nc.dram_tensor("wb", [128, 948], BF16, kind="ExternalInput")
    out = nc.dram_tensor("out", [N, N], F32, kind="ExternalOutput")

    out_sb = nc.alloc_sbuf_tensor("out_sbuf", [N, N], F32)
    out_sem = nc.alloc_semaphore("out_dma_sem")

    with ExitStack() as ctx:
        tc = ctx.enter_context(tile.TileContext(nc))
        sb = ctx.enter_context(tc.tile_pool(name="sb", bufs=1))
        ps1 = ctx.enter_context(tc.tile_pool(name="ps1", bufs=3, space="PSUM"))
        ps2 = ctx.enter_context(tc.tile_pool(name="ps2", bufs=5, space="PSUM"))

        # ---- input DMAs: sync ring, in need-order ----
        wpk_sb = sb.tile([128, 466], BF16, tag="wpk")
        nc.sync.dma_start(out=wpk_sb, in_=wpk[:, :])
        w2k1_sb = sb.tile([13, 3, D2], BF16, tag="w2k1")
        nc.sync.dma_start(out=w2k1_sb, in_=w2k1[:, :, :])
        wb_sb = sb.tile([128, 948], BF16, tag="wb")
        nc.sync.dma_start(out=wb_sb, in_=wb[:, :])

        adjp = wpk_sb[0:36, 0:36]
        aux_v = wpk_sb[:, 456:466].bitcast(F32)  # [128, 5] peep/2 scalars

        # warm-up sigmoid (no DMA deps): pins the single act-table load
        # (sigmoid set, which covers tanh) into the DMA shadow.
        warm_src = sb.tile([1, 1], F32, tag="warm_src")
        nc.vector.memset(warm_src[:, :], 0.25)
        warm = sb.tile([1, 1], F32, tag="warm")
        nc.scalar.activation(warm[0:1, 0:1], warm_src[0:1, 0:1], AF.Sigmoid)

        # ones rows for the bias folds + zero-fill for the garbage rows
        # the merged whole-width tanhs read.
        x1T = sb.tile([128, 72], BF16, tag="x1T")
        nc.vector.memset(x1T[0:13, 36:72], 1.0)
        x2T = sb.tile([128, 108], BF16, tag="x2T")
        nc.vector.memset(x2T[0:25, 72:108], 1.0)
        cp1 = sb.tile([128, 72], BF16, tag="cp1")    # C' = 2C, layer 1
        nc.vector.memset(cp1[0:128, 36:72], 0.0)
        cp2 = sb.tile([128, 108], BF16, tag="cp2")   # C' = 2C, layer 2
        nc.vector.memset(cp2[0:128, 72:108], 0.0)

        # ---- layer 1: psum banks grouped for fine-grained waits ----
        # p1a = [Ic0|Tc0]; p1b = [Ic1|Tc1|Oc1] (12 rows); p1o0 = Oc0.
        p1a = ps1.tile([128, 72], F32, tag="ps1", name="p1a")
        p1b = ps1.tile([12, 108], F32, tag="ps1", name="p1b")
        p1o0 = ps1.tile([128, 36], F32, tag="ps1", name="p1o0")
        l1_lhs = (
            lambda a, b: wpk_sb[0:36, 36 + a : 36 + b],   # I/2
            lambda a, b: wpk_sb[0:36, 176 + a : 176 + b],  # T
            lambda a, b: wpk_sb[0:36, 316 + a : 316 + b],  # O
        )
        l1_dsts = (  # (chunk, gate) -> psum region
            (p1a[0:128, 0:36], p1a[0:128, 36:72], p1o0[0:128, 0:36]),
            (p1b[0:12, 0:36], p1b[0:12, 36:72], p1b[0:12, 72:108]),
        )
        for ci, (a, b) in enumerate(C1):
            for g in (0, 1, 2):
                nc.tensor.matmul(
                    l1_dsts[ci][g],
                    lhsT=l1_lhs[g](a, b), rhs=adjp,
                    start=True, stop=True,
                )
        # ACT: t = tanh([aI/2 | aT]) per chunk -> go1 c0 -> tanh(C) -> go1 c1
        t1 = sb.tile([128, 144], BF16, tag="t1")
        nc.scalar.activation(t1[0:128, 0:72], p1a[:, :], AF.Tanh)
        nc.scalar.activation(t1[0:12, 72:144], p1b[0:12, 0:72], AF.Tanh)
        # DVE per chunk: C' = (1+tI)*tT then po = (p/2)*C' + aO
        po1 = sb.tile([128, 72], BF16, tag="po1")
        p1o_regions = (p1o0[0:128, 0:36], p1b[0:12, 72:108])
        for ci, (cs, tcol) in enumerate(((128, 0), (12, 72))):
            col = ci * 36
            nc.vector.scalar_tensor_tensor(
                cp1[0:cs, col : col + 36], in0=t1[0:cs, tcol : tcol + 36],
                scalar=1.0, in1=t1[0:cs, tcol + 36 : tcol + 72],
                op0=OP.add, op1=OP.mult,
            )
            nc.vector.scalar_tensor_tensor(
                po1[0:cs, col : col + 36],
                in0=cp1[0:cs, col : col + 36],
                scalar=aux_v[0:cs, ci : ci + 1],
                in1=p1o_regions[ci],
                op0=OP.mult, op1=OP.add,
            )
        go1 = sb.tile([128, 72], BF16, tag="go1")
        nc.scalar.activation(go1[0:128, 0:36], po1[0:128, 0:36], AF.Sigmoid)
        tc1 = sb.tile([128, 72], BF16, tag="tc1")
        nc.scalar.activation(tc1, cp1[:, :], AF.Tanh, scale=0.5)
        nc.scalar.activation(go1[0:12, 36:72], po1[0:12, 36:72], AF.Sigmoid)
        # x1T = max(tanh(C), 0) * O
        nc.vector.scalar_tensor_tensor(
            x1T[0:128, 0:36], in0=tc1[0:128, 0:36], scalar=0.0,
            in1=go1[0:128, 0:36], op0=OP.max, op1=OP.mult,
        )
        nc.vector.scalar_tensor_tensor(
            x1T[0:12, 36:72], in0=tc1[0:12, 36:72], scalar=0.0,
            in1=go1[0:12, 36:72], op0=OP.max, op1=OP.mult,
        )

        # ---- layer 2: psum banks grouped for fine-grained waits ----
        # p2a/p2b/p2c = [I|T] per chunk; p2o01 = [Oc0|Oc1]; p2o2 = Oc2.
        p2it = (
            ps2.tile([128, 72], F32, tag="ps2", name="p2a"),
            ps2.tile([128, 72], F32, tag="ps2", name="p2b"),
            ps2.tile([24, 72], F32, tag="ps2", name="p2c"),
        )
        p2o01 = ps2.tile([128, 72], F32, tag="ps2", name="p2o01")
        p2o2 = ps2.tile([24, 36], F32, tag="ps2", name="p2o2")
        p2o_regions = (
            p2o01[0:128, 0:36], p2o01[0:128, 36:72], p2o2[0:24, 0:36]
        )
        w2k0 = (wb_sb[:, 0:280], wb_sb[:, 280:560], wb_sb[:, 560:840])
        for ci, (a, b) in enumerate(C2):
            cs = b - a
            for g, dst in (
                (0, p2it[ci][0:cs, 0:36]),
                (1, p2it[ci][0:cs, 36:72]),
                (2, p2o_regions[ci]),
            ):
                nc.tensor.matmul(
                    dst,
                    lhsT=w2k0[g][:, a:b], rhs=x1T[0:128, 0:36],
                    start=True, stop=False,
                )
                nc.tensor.matmul(
                    dst,
                    lhsT=w2k1_sb[0:13, g, a:b], rhs=x1T[0:13, 36:72],
                    start=False, stop=True,
                )
        t2 = sb.tile([128, 216], BF16, tag="t2")
        po2 = sb.tile([128, 108], BF16, tag="po2")
        for ci, cs in ((0, 128), (1, 128), (2, 24)):
            nc.scalar.activation(
                t2[0:cs, ci * 72 : ci * 72 + 72], p2it[ci][:, :], AF.Tanh
            )
        for ci, cs in ((0, 128), (1, 128), (2, 24)):
            col = ci * 36
            nc.vector.scalar_tensor_tensor(
                cp2[0:cs, col : col + 36],
                in0=t2[0:cs, 2 * col : 2 * col + 36], scalar=1.0,
                in1=t2[0:cs, 2 * col + 36 : 2 * col + 72],
                op0=OP.add, op1=OP.mult,
            )
            nc.vector.scalar_tensor_tensor(
                po2[0:cs, col : col + 36],
                in0=cp2[0:cs, col : col + 36],
                scalar=aux_v[0:cs, 2 + ci : 3 + ci],
                in1=p2o_regions[ci],
                op0=OP.mult, op1=OP.add,
            )
        tc2 = sb.tile([128, 108], BF16, tag="tc2")
        nc.scalar.activation(tc2, cp2[:, :], AF.Tanh, scale=0.5)
        go2 = sb.tile([128, 108], BF16, tag="go2")
        nc.scalar.activation(go2[0:128, 0:72], po2[0:128, 0:72], AF.Sigmoid)
        nc.scalar.activation(go2[0:24, 72:108], po2[0:24, 72:108], AF.Sigmoid)
        nc.vector.scalar_tensor_tensor(
            x2T[0:128, 0:72], in0=tc2[0:128, 0:72], scalar=0.0,
            in1=go2[0:128, 0:72], op0=OP.max, op1=OP.mult,
        )
        nc.vector.scalar_tensor_tensor(
            x2T[0:24, 72:108], in0=tc2[0:24, 72:108], scalar=0.0,
            in1=go2[0:24, 72:108], op0=OP.max, op1=OP.mult,
        )
        psfc = ps1.tile([N, 36], F32, tag="ps1", name="psfc")
        nc.tensor.matmul(
            psfc, lhsT=x2T[0:128, 0:35], rhs=wb_sb[:, 840:876],
            start=True, stop=False,
        )
        nc.tensor.matmul(
            psfc, lhsT=x2T[0:128, 36:71], rhs=wb_sb[:, 876:912],
            start=False, stop=False,
        )
        nc.tensor.matmul(
            psfc, lhsT=x2T[0:25, 72:107], rhs=wb_sb[0:25, 912:948],
            start=False, stop=True,
        )
        nc.vector.tensor_scalar_max(out_sb[0:N, 0:N], psfc[:, 0:N], 0.0)

    # Output DMA after the tile context, on the warm sync ring.  Fire
    # and forget: its flight hides under the compiler's fixed sem-reset
    # epilogue; ordering comes from the context-exit barrier.
    nc.sync.dma_start(out=out[:, :], in_=out_sb[0:N, 0:N]).then_inc(out_sem, 16)

    nc.compile()
    return nc


def pack_inputs(
    adj_matrix, W1, cheb1_b, peep1, b1, W2, cheb2_b, peep2, b2, fc_w, fc_b
) -> dict:
    """Host-side weight packing: gather/concat + bias fold + bf16 cast.

    The I-gate weights (and biases) of both layers and the peephole
    scalars are halved: the kernel computes I via tanh(aI/2)."""
    f = np.float32

    def gate_blk(Wg, bias, scale=1.0):  # [k+1, d] with the bias fold row
        blk = np.concatenate([Wg, bias[None, :]], axis=0) * scale
        return blk.astype(BF)

    adjp = np.zeros((36, 36), dtype=f)
    adjp[0:35, 0:35] = adj_matrix
    adjp[35, 0:35] = 1.0

    wpk_h = np.zeros((128, 466), dtype=BF)
    wpk_h[0:36, 0:36] = adjp.astype(BF)
    wpk_h[0:36, 36:176] = gate_blk(W1[0], cheb1_b[0] + b1[0], 0.5)
    wpk_h[0:36, 176:316] = gate_blk(W1[2], cheb1_b[2] + b1[2])
    wpk_h[0:36, 316:456] = gate_blk(W1[3], cheb1_b[3] + b1[3])
    aux_h = np.zeros((128, 5), dtype=f)
    aux_h[:, 0] = peep1[2][0:128] * 0.5
    aux_h[0:12, 1] = peep1[2][128:140] * 0.5
    aux_h[:, 2] = peep2[2][0:128] * 0.5
    aux_h[:, 3] = peep2[2][128:256] * 0.5
    aux_h[0:24, 4] = peep2[2][256:280] * 0.5
    wpk_h[:, 456:466] = np.ascontiguousarray(aux_h).view(BF)

    scales = {0: 0.5, 2: 1.0, 3: 1.0}
    w2k1_h = np.stack(
        [gate_blk(W2[g][128:140], cheb2_b[g] + b2[g], scales[g]) for g in GATES],
        axis=1,
    )  # [13, 3, 280]

    wb_h = np.zeros((128, 948), dtype=BF)
    wb_h[:, 0:280] = (W2[0][0:128] * 0.5).astype(BF)
    wb_h[:, 280:560] = W2[2][0:128].astype(BF)
    wb_h[:, 560:840] = W2[3][0:128].astype(BF)
    fcx = np.concatenate([fc_w, fc_b[None, :]], axis=0)  # [281, 35]
    wb_h[:, 840:875] = fcx[0:128].astype(BF)
    wb_h[:, 876:911] = fcx[128:256].astype(BF)
    wb_h[0:25, 912:947] = fcx[256:281].astype(BF)

    return {
        "wpk": np.ascontiguousarray(wpk_h),
        "w2k1": np.ascontiguousarray(w2k1_h),
        "wb": np.ascontiguousarray(wb_h),
    }


_NC_CACHE: list = []


def kernel(
    adj_matrix,
    W1,
    cheb1_W,
    cheb1_b,
    peep1,
    b1,
    W2,
    cheb2_W,
    cheb2_b,
    peep2,
    b2,
    fc_w,
    fc_b,
) -> np.ndarray:
    from concourse.bass_utils import run_bass_kernel_spmd

    in_map = pack_inputs(
        adj_matrix, W1, cheb1_b, peep1, b1, W2, cheb2_b, peep2, b2, fc_w, fc_b
    )

    if not _NC_CACHE:
        _NC_CACHE.append(build_nc())
    nc = _NC_CACHE[0]

    in_maps = [dict(in_map) for _ in range(N_CORES)]
    try:
        res = run_bass_kernel_spmd(nc, in_maps, core_ids=list(range(N_CORES)))
    except Exception:
        # transient device wedges (NRT_EXEC_*) usually clear on re-run
        res = run_bass_kernel_spmd(nc, in_maps, core_ids=list(range(N_CORES)))
    return np.asarray(res.results[0]["out"], dtype=np.float32)


# revision 21
# speedup vs baseline: 1.1174x; 1.0453x over previous
"""Trainium2 Bass kernel for nn_GCLSTMModel_48868137894020 (v3).

Same algebraic reduction as v1/v2 (H0 = C0 = 0 kills the cheb convs, the
forget gate, and peep[0/1]); per layer (d = 140 then 280), X = input:
  I = sigmoid(X @ W[0] + cb[0] + b[0])
  T = tanh   (X @ W[2] + cb[2] + b[2])
  C = I * T
  O = sigmoid(X @ W[3] + cb[3] + b[3] + peep[2] * C)
  X' = relu(O * tanh(C)) == max(tanh(C), 0) * O
then out = relu(X'' @ fc_w + fc_b).

v3 = v2 layout (everything transposed: features on partitions, nodes on
the free dim; bf16 matmuls; biases folded via a ones row) with the DMA
plan rebuilt from the v2 trace:
  - v2 issued 6 weight DMAs on the single sync HWDGE ring; the L1 O-gate
    weights (DMA #2) only landed at ~11.8us and the whole L1 -> L2 chain
    was DMA-paced, not compute-paced.
  - v3 packs ALL L1 weights + adj + aux scalars into ONE [128, 466] bf16
    transfer on the sync ring, with w2k1 right behind it, and ALL L2 +
    FC weights into ONE [128, 948] bf16 transfer on the scalar HWDGE
    ring (qActDynamicHW) which issues in parallel.  Queueing the big
    L2 pack on the sync ring instead delays the L1 pack's final
    completion increment by 1-2us (straggler SDMA engine), so the
    two-ring split is load-bearing (v5/v6 measurements).
  - The ACT-queue order is: wb DMA issue first, then the activation
    warm-ups (whose auto-inserted table loads run in the DMA shadow).

Sharding: problem is tiny (N=35); all 8 cores run the identical program
on replicated inputs (no collectives), output taken from core 0.
"""

import sys

for _p in ("/opt/trn_rl_repo", "/opt/pypackages"):
    if _p not in sys.path:
        sys.path.append(_p)

from contextlib import ExitStack

import numpy as np
import ml_dtypes

import concourse.bacc as bacc
import concourse.bass as bass
import concourse.mybir as mybir
import concourse.tile as tile

F32 = mybir.dt.float32
BF16 = mybir.dt.bfloat16
AF = mybir.ActivationFunctionType
OP = mybir.AluOpType
GATES = (0, 2, 3)  # I, T (cell), O - forget gate (1) is dead
N = 35
D1 = 140
D2 = 280
N_CORES = 8
BF = ml_dtypes.bfloat16

# L1 feature chunks (140 = 128 + 12), L2 chunks (280 = 128 + 256 + 24).
C1 = ((0, 128), (128, 140))
C2 = ((0, 128), (128, 256), (256, 280))


def build_nc() -> bass.Bass:
    nc = bacc.Bacc()

    # wpk: [128, 466] bf16 - ALL layer-1 weights in one transfer:
    #   cols 0:36    rows 0:36 = [adj; ones] (k=36 rhs for L1)
    #   cols 36:176  rows 0:36 = [W1[I]; bias]
    #   cols 176:316 rows 0:36 = [W1[T]; bias]
    #   cols 316:456 rows 0:36 = [W1[O]; bias]
    #   cols 456:466 rows 0:128 = peep1[2]/peep2[2] per-partition
    #                scalars as a bit-cast fp32 [128, 5] block
    wpk = nc.dram_tensor("wpk", [128, 466], BF16, kind="ExternalInput")
    # W2 k-chunk1 (input features 128:140 + bias row): [13, 3, 280]
    w2k1 = nc.dram_tensor("w2k1", [13, 3, D2], BF16, kind="ExternalInput")
    # wb: [128, 948] bf16 - ALL layer-2 + FC weights (k-chunk0):
    #   cols 0:280 = W2[I], 280:560 = W2[T], 560:840 = W2[O],
    #   cols 840:948 = [fc_w; fc_b] row-chunks as [128, 3, 36]
    wb = nc.dram_tensor("wb", [128, 948], BF16, kind="ExternalInput")
    out = nc.dram_tensor("out", [N, N], F32, kind="ExternalOutput")

    # persistent sbuf tensor: written by the relu inside the tile
    # context, DMA'd out AFTER the context (ordered by the context-exit
    # all-engine barrier) with a semaphore nothing waits on.
    out_sb = nc.alloc_sbuf_tensor("out_sbuf", [N, N], F32)
    out_sem = nc.alloc_semaphore("out_dma_sem")

    with ExitStack() as ctx:
        tc = ctx.enter_context(tile.TileContext(nc))
        sb = ctx.enter_context(tc.tile_pool(name="sb", bufs=1))
        ps1 = ctx.enter_context(tc.tile_pool(name="ps1", bufs=3, space="PSUM"))
        ps2 = ctx.enter_context(tc.tile_pool(name="ps2", bufs=4, space="PSUM"))

        # ---- input DMAs: two HWDGE rings issue in parallel ----
        # wpk rides the sync ring ALONE: any big transfer queued behind
        # it on the same ring delays its final completion increment by
        # 1-2us (straggler SDMA engine, v5/v6 traces).  wb + w2k1 ride
        # the scalar ring.
        wpk_sb = sb.tile([128, 466], BF16, tag="wpk")
        nc.sync.dma_start(out=wpk_sb, in_=wpk[:, :])
        wb_sb = sb.tile([128, 948], BF16, tag="wb")
        nc.scalar.dma_start(out=wb_sb, in_=wb[:, :])
        w2k1_sb = sb.tile([13, 3, D2], BF16, tag="w2k1")
        nc.scalar.dma_start(out=w2k1_sb, in_=w2k1[:, :, :])

        adjp = wpk_sb[0:36, 0:36]
        aux_v = wpk_sb[:, 456:466].bitcast(F32)  # [128, 5] peep scalars

        # warm-up sigmoid with no DMA deps: forces the act-table load
        # into the DMA wait; its set also covers tanh (warming tanh too
        # just burns ACT time right before the first gate).
        warm_src = sb.tile([1, 1], F32, tag="warm_src")
        nc.vector.memset(warm_src[:, :], 0.25)
        warm = sb.tile([1, 1], F32, tag="warm")
        nc.scalar.activation(warm[0:1, 0:1], warm_src[0:1, 0:1], AF.Sigmoid)

        # ones rows for the bias folds: memset the whole chunk region
        # (partition offset must be 0-aligned); the gate STT later
        # overwrites all but the last row.
        x1T = sb.tile([128, 72], BF16, tag="x1T")
        nc.vector.memset(x1T[0:13, 36:72], 1.0)
        x2T = sb.tile([128, 108], BF16, tag="x2T")
        nc.vector.memset(x2T[0:25, 72:108], 1.0)

        # ---- layer 1 (transposed): psum banks [128, 72] ----
        p1 = {
            g: ps1.tile([128, 72], F32, tag="ps1", name=f"p1_{g}")
            for g in range(3)
        }
        l1_lhs = (
            lambda a, b: wpk_sb[0:36, 36 + a : 36 + b],
            lambda a, b: wpk_sb[0:36, 176 + a : 176 + b],
            lambda a, b: wpk_sb[0:36, 316 + a : 316 + b],
        )
        for g in range(3):
            for ci, (a, b) in enumerate(C1):
                nc.tensor.matmul(
                    p1[g][0 : b - a, ci * 36 : ci * 36 + 36],
                    lhsT=l1_lhs[g](a, b),
                    rhs=adjp,
                    start=True,
                    stop=True,
                )
        gi1 = sb.tile([128, 72], BF16, tag="gi1")
        nc.scalar.activation(gi1, p1[0][:, :], AF.Sigmoid)
        gt1 = sb.tile([128, 72], BF16, tag="gt1")
        nc.scalar.activation(gt1, p1[1][:, :], AF.Tanh)
        c1 = sb.tile([128, 72], BF16, tag="c1")
        nc.vector.tensor_mul(c1, gi1, gt1)
        tc1 = sb.tile([128, 72], BF16, tag="tc1")
        nc.scalar.activation(tc1, c1, AF.Tanh)
        po1 = sb.tile([128, 72], BF16, tag="po1")
        for ci, (a, b) in enumerate(C1):
            cs = b - a
            nc.vector.scalar_tensor_tensor(
                po1[0:cs, ci * 36 : ci * 36 + 36],
                in0=c1[0:cs, ci * 36 : ci * 36 + 36],
                scalar=aux_v[0:cs, ci : ci + 1],
                in1=p1[2][0:cs, ci * 36 : ci * 36 + 36],
                op0=OP.mult, op1=OP.add,
            )
        # O sigmoid split per chunk: chunk0's fires as soon as its STT
        # lands, so x1T chunk0 (the big L2 matmul rhs) is ready earlier
        go1 = sb.tile([128, 72], BF16, tag="go1")
        nc.scalar.activation(go1[0:128, 0:36], po1[0:128, 0:36], AF.Sigmoid)
        nc.scalar.activation(go1[0:12, 36:72], po1[0:12, 36:72], AF.Sigmoid)
        # x1T = max(tanh(C), 0) * O
        nc.vector.scalar_tensor_tensor(
            x1T[0:128, 0:36], in0=tc1[0:128, 0:36], scalar=0.0,
            in1=go1[0:128, 0:36], op0=OP.max, op1=OP.mult,
        )
        nc.vector.scalar_tensor_tensor(
            x1T[0:12, 36:72], in0=tc1[0:12, 36:72], scalar=0.0,
            in1=go1[0:12, 36:72], op0=OP.max, op1=OP.mult,
        )

        # ---- layer 2 (transposed): psum banks [128, 108] ----
        p2 = {
            g: ps2.tile([128, 108], F32, tag="ps2", name=f"p2_{g}")
            for g in range(3)
        }
        w2k0_views = (wb_sb[:, 0:280], wb_sb[:, 280:560], wb_sb[:, 560:840])
        # ONE open accumulation group per PSUM bank: close each region
        # (k0 then k1) before opening the next region in that bank.
        for g in (0, 1, 2):
            for ci, (a, b) in enumerate(C2):
                nc.tensor.matmul(
                    p2[g][0 : b - a, ci * 36 : ci * 36 + 36],
                    lhsT=w2k0_views[g][:, a:b],
                    rhs=x1T[0:128, 0:36],
                    start=True,
                    stop=False,
                )
                nc.tensor.matmul(
                    p2[g][0 : b - a, ci * 36 : ci * 36 + 36],
                    lhsT=w2k1_sb[0:13, g, a:b],
                    rhs=x1T[0:13, 36:72],
                    start=False,
                    stop=True,
                )
        # every gate stage split into an A piece (chunks 0+1, ready one
        # third of the MM stream earlier) and a small c2 piece, so the
        # gate chain overlaps the tail of the matmul stream
        A = (slice(0, 128), slice(0, 72))
        B = (slice(0, 24), slice(72, 108))
        gi2 = sb.tile([128, 108], BF16, tag="gi2")
        nc.scalar.activation(gi2[A], p2[0][A], AF.Sigmoid)
        nc.scalar.activation(gi2[B], p2[0][B], AF.Sigmoid)
        gt2 = sb.tile([128, 108], BF16, tag="gt2")
        nc.scalar.activation(gt2[A], p2[1][A], AF.Tanh)
        nc.scalar.activation(gt2[B], p2[1][B], AF.Tanh)
        c2 = sb.tile([128, 108], BF16, tag="c2")
        nc.vector.tensor_mul(c2[A], gi2[A], gt2[A])
        nc.vector.tensor_mul(c2[B], gi2[B], gt2[B])
        tc2 = sb.tile([128, 108], BF16, tag="tc2")
        nc.scalar.activation(tc2[A], c2[A], AF.Tanh)
        nc.scalar.activation(tc2[B], c2[B], AF.Tanh)
        po2 = sb.tile([128, 108], BF16, tag="po2")
        for ci, (a, b) in enumerate(C2):
            cs = b - a
            nc.vector.scalar_tensor_tensor(
                po2[0:cs, ci * 36 : ci * 36 + 36],
                in0=c2[0:cs, ci * 36 : ci * 36 + 36],
                scalar=aux_v[0:cs, 2 + ci : 3 + ci],
                in1=p2[2][0:cs, ci * 36 : ci * 36 + 36],
                op0=OP.mult, op1=OP.add,
            )
        go2 = sb.tile([128, 108], BF16, tag="go2")
        nc.scalar.activation(go2[A], po2[A], AF.Sigmoid)
        nc.scalar.activation(go2[B], po2[B], AF.Sigmoid)
        nc.vector.scalar_tensor_tensor(
            x2T[A], in0=tc2[A], scalar=0.0, in1=go2[A],
            op0=OP.max, op1=OP.mult,
        )
        nc.vector.scalar_tensor_tensor(
            x2T[B], in0=tc2[B], scalar=0.0, in1=go2[B],
            op0=OP.max, op1=OP.mult,
        )
        psfc = ps2.tile([N, 36], F32, tag="ps2", name="psfc")
        nc.tensor.matmul(
            psfc, lhsT=x2T[0:128, 0:35], rhs=wb_sb[:, 840:876],
            start=True, stop=False,
        )
        nc.tensor.matmul(
            psfc, lhsT=x2T[0:128, 36:71], rhs=wb_sb[:, 876:912],
            start=False, stop=False,
        )
        nc.tensor.matmul(
            psfc, lhsT=x2T[0:25, 72:107], rhs=wb_sb[0:25, 912:948],
            start=False, stop=True,
        )
        nc.vector.tensor_scalar_max(out_sb[0:N, 0:N], psfc[:, 0:N], 0.0)

    # Output DMA after the tile context. Fire-and-forget: the ~2us
    # transfer finishes in the shadow of the compiler's fixed ~6.5us
    # semaphore-reset epilogue that runs after the last instruction, so
    # no engine ever waits on its completion.
    nc.sync.dma_start(out=out[:, :], in_=out_sb[0:N, 0:N]).then_inc(out_sem, 16)

    nc.compile()
    return nc


def pack_inputs(
    adj_matrix, W1, cheb1_b, peep1, b1, W2, cheb2_b, peep2, b2, fc_w, fc_b
) -> dict:
    """Host-side weight packing: gather/concat + bias fold + bf16 cast."""
    f = np.float32

    def gate_blk(Wg, bias):  # [k+1, d] with the bias fold row
        return np.concatenate([Wg, bias[None, :]], axis=0).astype(BF)

    adjp = np.zeros((36, 36), dtype=f)
    adjp[0:35, 0:35] = adj_matrix
    adjp[35, 0:35] = 1.0

    wpk_h = np.zeros((128, 466), dtype=BF)
    wpk_h[0:36, 0:36] = adjp.astype(BF)
    wpk_h[0:36, 36:176] = gate_blk(W1[0], cheb1_b[0] + b1[0])
    wpk_h[0:36, 176:316] = gate_blk(W1[2], cheb1_b[2] + b1[2])
    wpk_h[0:36, 316:456] = gate_blk(W1[3], cheb1_b[3] + b1[3])
    aux_h = np.zeros((128, 5), dtype=f)
    aux_h[:, 0] = peep1[2][0:128]
    aux_h[0:12, 1] = peep1[2][128:140]
    aux_h[:, 2] = peep2[2][0:128]
    aux_h[:, 3] = peep2[2][128:256]
    aux_h[0:24, 4] = peep2[2][256:280]
    wpk_h[:, 456:466] = np.ascontiguousarray(aux_h).view(BF)

    w2k1_h = np.stack(
        [gate_blk(W2[g][128:140], cheb2_b[g] + b2[g]) for g in GATES],
        axis=1,
    )  # [13, 3, 280]

    wb_h = np.zeros((128, 948), dtype=BF)
    wb_h[:, 0:280] = W2[0][0:128].astype(BF)
    wb_h[:, 280:560] = W2[2][0:128].astype(BF)
    wb_h[:, 560:840] = W2[3][0:128].astype(BF)
    fcx = np.concatenate([fc_w, fc_b[None, :]], axis=0)  # [281, 35]
    wb_h[:, 840:875] = fcx[0:128].astype(BF)
    wb_h[:, 876:911] = fcx[128:256].astype(BF)
    wb_h[0:25, 912:947] = fcx[256:281].astype(BF)

    return {
        "wpk": np.ascontiguousarray(wpk_h),
        "w2k1": np.ascontiguousarray(w2k1_h),
        "wb": np.ascontiguousarray(wb_h),
    }


_NC_CACHE: list = []


def kernel(
    adj_matrix,
    W1,
    cheb1_W,
    cheb1_b,
    peep1,
    b1,
    W2,
    cheb2_W,
    cheb2_b,
    peep2,
    b2,
    fc_w,
    fc_b,
) -> np.ndarray:
    from concourse.bass_utils import run_bass_kernel_spmd

    in_map = pack_inputs(
        adj_matrix, W1, cheb1_b, peep1, b1, W2, cheb2_b, peep2, b2, fc_w, fc_b
    )

    if not _NC_CACHE:
        _NC_CACHE.append(build_nc())
    nc = _NC_CACHE[0]

    in_maps = [dict(in_map) for _ in range(N_CORES)]
    try:
        res = run_bass_kernel_spmd(nc, in_maps, core_ids=list(range(N_CORES)))
    except Exception:
        # transient device wedges (NRT_EXEC_*) usually clear on re-run
        res = run_bass_kernel_spmd(nc, in_maps, core_ids=list(range(N_CORES)))
    return np.asarray(res.results[0]["out"], dtype=np.float32)
